# revision 4
# baseline (speedup 1.0000x reference)
"""TRN2 Bass kernel for nn_Blur: depthwise 4x4 FIR blur (stylegan2 upfirdn2d).

out[n,c,h,w] = sum_{i,j} wflip[i,j] * x[n,c,h+i-1,w+j-1]   (zero-padded)

Strategy (per NeuronCore, 8-way data parallel over the 512 (n,c) images):
  - Layout: image rows on SBUF partitions, row pixels on the free dim.
  - The 2D conv runs entirely on the TensorEngine as 4 PSUM-accumulated
    matmuls (one per horizontal tap j), each contracting over input rows
    with a small banded matrix encoding the vertical taps; the horizontal
    shift is just a free-dim AP offset on the moving operand.
  - Precision: hi/lo float32r (TF32) split. x = hi + lo with hi/lo rounded
    to TF32; band weights are exactly representable in TF32, PSUM
    accumulates in fp32, so the only error is the lost tail of lo
    (~2^-22 relative) -> ~1e-6 overall.
  - 8 matmuls per (image, row-block): 4 taps x {hi, lo}.

Self-contained: hardcodes shapes from the problem spec.
"""

import numpy as np
from contextlib import ExitStack

import concourse.bacc as bacc
import concourse.mybir as mybir
import concourse.tile as tile
from concourse.bass_utils import run_bass_kernel_spmd

N_IMG, C, H, W = 4, 128, 513, 513
OH = OW = 512
NCORES = 8
IPC = (N_IMG * C) // NCORES  # 64 images per core

# (h0, BM, r0, Kb): output rows [h0, h0+BM) from input rows [r0, r0+Kb)
BLOCKS = [
    (0, 125, 0, 127),
    (125, 125, 124, 128),
    (250, 125, 249, 128),
    (375, 125, 374, 128),
    (500, 12, 499, 14),
]
# SBUF x tiles are zero-padded in w: tile col t <-> x col (t-1), cols 0,
# 514, 515 are zeros (fp32r matmul needs even N, so every tap runs full
# N=512: tap j reads tile cols [j, j+512)).
XW = 516
XBUFS = 12

SPLIT = True  # hi/lo TF32 split (8 MMs) vs plain TF32 (4 MMs)

TRACE = False  # set by test harness for profiling runs
LAST_RESULTS = None  # BassKernelResults of the last run (for profiling)

_CACHE = {}


def _make_bands_np(kernel):
    """Pack the 20 banded matrices into one [128, 20*128] f32 array."""
    wflip = np.flip(np.asarray(kernel, dtype=np.float32), (0, 1)).astype(np.float64)
    bands = np.zeros((128, len(BLOCKS) * 4 * 128), dtype=np.float32)
    for bi, (h0, BM, r0, Kb) in enumerate(BLOCKS):
        for j in range(4):
            col0 = (bi * 4 + j) * 128
            # A[k, m] = wflip[(r0+k)-(h0+m)+1, j] for valid vertical tap
            k_idx = np.arange(Kb)[:, None]
            m_idx = np.arange(BM)[None, :]
            i_idx = (r0 + k_idx) - (h0 + m_idx) + 1
            valid = (i_idx >= 0) & (i_idx < 4)
            A = np.where(valid, wflip[np.clip(i_idx, 0, 3), j], 0.0)
            bands[:Kb, col0 : col0 + BM] = A.astype(np.float32)
    return bands


def _build():
    f32 = mybir.dt.float32
    f32r = mybir.dt.float32r
    nc = bacc.Bacc("TRN2", target_bir_lowering=False, debug=False)

    x_d = nc.dram_tensor("x", [IPC, H, W], f32, kind="ExternalInput").ap()
    bands_d = nc.dram_tensor("bands", [128, 20 * 128], f32, kind="ExternalInput").ap()
    out_d = nc.dram_tensor("out", [IPC, OH, OW], f32, kind="ExternalOutput").ap()

    with tile.TileContext(nc) as tc, ExitStack() as ctx:
        cpool = ctx.enter_context(tc.tile_pool(name="const", bufs=1))
        xpool = ctx.enter_context(tc.tile_pool(name="x", bufs=XBUFS))
        hipool = ctx.enter_context(tc.tile_pool(name="hi", bufs=4))
        lopool = ctx.enter_context(tc.tile_pool(name="lo", bufs=4))
        opool = ctx.enter_context(tc.tile_pool(name="o", bufs=6))
        pspool = ctx.enter_context(tc.tile_pool(name="ps", bufs=6, space="PSUM"))

        # Bands: DMA as f32, round once to TF32.
        bands_f32 = cpool.tile([128, 20 * 128], f32, tag="bf")
        nc.sync.dma_start(bands_f32[:], bands_d[:])
        bands_sb = cpool.tile([128, 20 * 128], f32r, tag="br")
        nc.vector.tensor_copy(bands_sb[:], bands_f32[:])

        it = 0
        for img in range(IPC):
            for bi, (h0, BM, r0, Kb) in enumerate(BLOCKS):
                xt = xpool.tile([128, XW], f32, tag="xt")
                if it < XBUFS:
                    # Zero the pad columns once per physical buffer; later
                    # generations reuse the same slots and never overwrite
                    # these columns.
                    nc.gpsimd.memset(xt[:, 0:1], 0.0)
                    nc.gpsimd.memset(xt[:, 514:516], 0.0)
                nc.sync.dma_start(xt[0:Kb, 1:514], x_d[img, r0 : r0 + Kb, :])

                hi = hipool.tile([128, XW], f32r, tag="hi")
                nc.vector.tensor_copy(hi[0:Kb, :], xt[0:Kb, :])
                if SPLIT:
                    lo = lopool.tile([128, XW], f32r, tag="lo")
                    nc.vector.tensor_sub(lo[0:Kb, :], xt[0:Kb, :], hi[0:Kb, :])
                    parts = (hi, lo)
                else:
                    parts = (hi,)

                p = pspool.tile([128, OW], f32, tag="p")
                n_mm = 4 * len(parts)
                g = 0
                for part in parts:
                    for j in range(4):
                        col0 = (bi * 4 + j) * 128
                        nc.tensor.matmul(
                            p[0:BM, :],
                            bands_sb[0:Kb, col0 : col0 + BM],
                            part[0:Kb, j : j + 512],
                            start=(g == 0),
                            stop=(g == n_mm - 1),
                        )
                        g += 1

                ot = opool.tile([128, OW], f32, tag="ot")
                nc.scalar.copy(ot[0:BM, :], p[0:BM, :])
                nc.sync.dma_start(out_d[img, h0 : h0 + BM, :], ot[0:BM, :])
                it += 1

    nc.compile()
    return nc


def kernel(input, kernel):
    global LAST_RESULTS
    x = np.ascontiguousarray(np.asarray(input, dtype=np.float32))
    kern = np.asarray(kernel, dtype=np.float32)
    assert x.shape == (N_IMG, C, H, W), x.shape

    if "nc" not in _CACHE:
        _CACHE["nc"] = _build()
    nc = _CACHE["nc"]

    bands = _make_bands_np(kern)
    x_flat = x.reshape(N_IMG * C, H, W)
    in_maps = [
        {"x": x_flat[k * IPC : (k + 1) * IPC], "bands": bands} for k in range(NCORES)
    ]
    res = run_bass_kernel_spmd(nc, in_maps, list(range(NCORES)), trace=TRACE)
    LAST_RESULTS = res

    out = np.concatenate([res.results[k]["out"] for k in range(NCORES)], axis=0)
    return out.reshape(N_IMG, C, OH, OW)
